# revision 24
# baseline (speedup 1.0000x reference)
"""MultiHeadLatentAttention on 8 trn2 NeuronCores (Bass/Tile).

Sharding: core = (b, qc) with b = core//4 (data parallel over batch),
qc = core%4 (query-chunk of 512 positions). Each core computes the full
K/V for its batch (replicated within the 4-core batch group) and the
attention + output projection for its 512 queries; the host gather is a
pure concatenation.

v2 design relative to the first working kernel:
  - rope scores computed once per key-chunk (shared across heads) and
    added into score PSUM via DVE scalar_tensor_tensor for 14/16 key
    chunks; 2 chunks keep the PE rope-matmul path (engine balancing).
  - phase B is head-group-outer (4 groups x 4 heads): k_c/v_c produced
    per group over the full key range; ctx accumulates in PSUM across
    all 16 key chunks (no SBUF fp32 ctx accumulator, no DVE adds).
  - softmax sums accumulate in one packed PSUM bank (4 heads at
    partitions 0/32/64/96 via explicit matmul tile_position); one
    strided copy + reciprocal_approx_fast per group.
  - b_UV folded into b_O on the host (bo_eff = b_O + b_UV @ W_O).
  - W_DKV/W_KR/W_UK/W_UV + hTq + rope tables + consts are loaded once
    (rep 0) and stay resident; W_DQ/W_QR/W_UQ stream per rep in phase A;
    W_O streams with DMAs issued at the start of phase B.
  - exp batched over key-chunk pairs; softmax-sum adds split DVE/gpsimd.
"""
import math
import numpy as np
from contextlib import ExitStack

import concourse.bass as bass
import concourse.mybir as mybir
import concourse.tile as tile
from concourse.bass_utils import run_bass_kernel_spmd

F32R = mybir.dt.float32r
F32 = mybir.dt.float32
F16 = mybir.dt.float16
AF = mybir.ActivationFunctionType
ALU = mybir.AluOpType

H, NH, LAT = 2048, 16, 512
D = H // NH            # 128
B, S = 2, 2048
SQ = S // 4            # 512 queries per core
E = H // 128           # 16 contraction chunks over H
L = LAT // 128         # 4 chunks over LAT
G = 4                  # head groups in phase B
HG = NH // G           # heads per group
KS = S // 128          # 16 key chunks of 128
SCALE = 1.0 / math.sqrt(D)
SHIFT = 8.0            # softmax logit shift (cancels in normalization)
XKS = (0, 3, 6, 9, 12, 15)   # key chunks whose rope add runs on the PE

_n_split = 0


def _split_multi_waits(nc):
    """walrus in this container allows ONE sync wait per instruction; Tile
    attaches several. Hoist extras onto single-wait NoOps just before."""
    global _n_split
    total = 0
    for f in nc.m.functions:
        for bb in f.blocks:
            out = []
            changed = False
            for inst in bb.instructions:
                si = inst.sync_info
                if si is not None and len(si.on_wait) > 1:
                    changed = True
                    waits = list(si.on_wait)
                    for w in waits[:-1]:
                        _n_split += 1
                        total += 1
                        nop = mybir.InstNoOp(
                            name=f"wsplit-{_n_split}", text_hint="waitsplit")
                        nop.engine = inst.engine
                        nop.sync_info = mybir.SyncInfo(on_wait=[w], on_update=[])
                        nc.register_instruction(nop)
                        out.append(nop)
                    inst.sync_info = mybir.SyncInfo(
                        on_wait=[waits[-1]], on_update=list(si.on_update))
                out.append(inst)
            if changed:
                bb.instructions = out
    return total


def _strided_partitions(ap, step, n):
    """AP over the same tensor reading partitions 0, step, 2*step, ..."""
    return bass.AP(tensor=ap.tensor, offset=ap.offset,
                   ap=[[step, n]] + [list(p) for p in ap.ap[1:]])


def _bcast_partitions(ap, n=128):
    """AP broadcasting a DRAM region (all dims kept) across n partitions."""
    return bass.AP(tensor=ap.tensor, offset=ap.offset,
                   ap=[[0, n]] + [list(p) for p in ap.ap])


def build(reps=1):
    nc = bass.Bass()

    d_hT = nc.dram_tensor("hT", [H, S], F16, kind="ExternalInput")
    d_hTq = nc.dram_tensor("hTq", [H, SQ], F16, kind="ExternalInput")
    d_wdkv = nc.dram_tensor("wdkv", [H, LAT], F16, kind="ExternalInput")
    d_wdq = nc.dram_tensor("wdq", [H, LAT], F16, kind="ExternalInput")
    d_wkr = nc.dram_tensor("wkr", [H, D], F16, kind="ExternalInput")
    d_wqr = nc.dram_tensor("wqr", [H, D], F16, kind="ExternalInput")
    d_wuk = nc.dram_tensor("wuk", [LAT, H], F16, kind="ExternalInput")
    d_wuv = nc.dram_tensor("wuv", [LAT, H], F16, kind="ExternalInput")
    d_wuq = nc.dram_tensor("wuq", [LAT, H], F16, kind="ExternalInput")
    d_wo = nc.dram_tensor("wo", [H, H], F16, kind="ExternalInput")
    d_cos2 = nc.dram_tensor("cos2", [D, S], F16, kind="ExternalInput")
    d_sins = nc.dram_tensor("sins", [D, S], F16, kind="ExternalInput")
    d_cos2q = nc.dram_tensor("cos2q", [D, SQ], F16, kind="ExternalInput")
    d_sinsq = nc.dram_tensor("sinsq", [D, SQ], F16, kind="ExternalInput")
    d_maskb = nc.dram_tensor("maskb", [128, E], F32, kind="ExternalInput")
    d_bdkv = nc.dram_tensor("bdkv", [128, L], F32, kind="ExternalInput")
    d_bdq = nc.dram_tensor("bdq", [128, L], F32, kind="ExternalInput")
    d_buk = nc.dram_tensor("buk", [128, NH], F32, kind="ExternalInput")
    d_buq = nc.dram_tensor("buq", [128, NH], F32, kind="ExternalInput")
    d_bkr = nc.dram_tensor("bkr", [128, 1], F32, kind="ExternalInput")
    d_bqr = nc.dram_tensor("bqr", [128, 1], F32, kind="ExternalInput")
    d_bo = nc.dram_tensor("bo", [1, H], F16, kind="ExternalInput")
    d_sel4 = nc.dram_tensor("sel4", [128, 16], F16, kind="ExternalInput")
    d_selb = nc.dram_tensor("selb", [4, HG * 128], F16, kind="ExternalInput")
    d_out = nc.dram_tensor("out", [SQ, H], F32, kind="ExternalOutput")

    with tile.TileContext(nc) as tc, ExitStack() as es:
        perm = es.enter_context(tc.tile_pool(name="perm", bufs=1))

        # resident weights + activations (per-partition KB in comments)
        wdkv = perm.tile([128, E, LAT], F16)       # 16
        wkr = perm.tile([128, E, D], F16)          # 4
        wuk = perm.tile([128, L, H], F16)          # 16
        wuv = perm.tile([128, L, H], F16)          # 16
        htq = perm.tile([128, E, SQ], F16)         # 16
        ckvT = perm.tile([128, L, S], F16)         # 16
        qcT = perm.tile([128, NH, SQ], F16)        # 16
        ctxf = perm.tile([128, NH, SQ], F16)       # 16
        krro = perm.tile([128, S], F16)            # 4
        qrro = perm.tile([128, SQ], F16)           # 1
        ropeS = perm.tile([128, KS, SQ], F16)      # 16

        maskb = perm.tile([128, E], F32)
        bdkv = perm.tile([128, L], F32)
        bdq = perm.tile([128, L], F32)
        buk = perm.tile([128, NH], F32)
        buq = perm.tile([128, NH], F32)
        bkr = perm.tile([128, 1], F32)
        bqr = perm.tile([128, 1], F32)
        bo_b = perm.tile([128, H], F16)            # 4 (bias broadcast)
        sel4 = perm.tile([128, HG, HG], F16)
        selb = perm.tile([4, HG, 128], F16)

        for _rep in range(reps):
            if _rep == 0:
                nc.sync.dma_start(
                    out=wdkv[:], in_=d_wdkv.rearrange("(e p) l -> p e l", p=128))
                nc.sync.dma_start(
                    out=wkr[:], in_=d_wkr.rearrange("(e p) d -> p e d", p=128))
                nc.sync.dma_start(
                    out=wuk[:], in_=d_wuk.rearrange("(l p) h -> p l h", p=128))
                nc.sync.dma_start(
                    out=wuv[:], in_=d_wuv.rearrange("(l p) h -> p l h", p=128))
                nc.gpsimd.dma_start(
                    out=htq[:], in_=d_hTq.rearrange("(e p) q -> p e q", p=128))
                for t, d in [(maskb, d_maskb),
                             (bdkv, d_bdkv), (bdq, d_bdq), (buk, d_buk),
                             (buq, d_buq), (bkr, d_bkr), (bqr, d_bqr),
                             (sel4, d_sel4), (selb, d_selb)]:
                    nc.gpsimd.dma_start(out=t[:], in_=d[:])
                nc.gpsimd.dma_start(out=bo_b[:], in_=_bcast_partitions(d_bo[0:1, :]))

            # ============ Phase A: projections + rope scores ============
            with tc.tile_pool(name="stA", bufs=1) as stA, \
                 tc.tile_pool(name="psA", bufs=1, space="PSUM") as psA:
                wdq = stA.tile([128, E, LAT], F16)
                wqr = stA.tile([128, E, D], F16)
                wuq = stA.tile([128, L, H], F16)
                nc.gpsimd.dma_start(
                    out=wdq[:], in_=d_wdq.rearrange("(e p) l -> p e l", p=128))
                nc.gpsimd.dma_start(
                    out=wqr[:], in_=d_wqr.rearrange("(e p) d -> p e d", p=128))
                nc.gpsimd.dma_start(
                    out=wuq[:], in_=d_wuq.rearrange("(l p) h -> p l h", p=128))
                cos2 = stA.tile([128, S], F16)
                sins = stA.tile([128, S], F16)
                cos2q = stA.tile([128, SQ], F16)
                sinsq = stA.tile([128, SQ], F16)
                for t, d in [(cos2, d_cos2), (sins, d_sins),
                             (cos2q, d_cos2q), (sinsq, d_sinsq)]:
                    nc.gpsimd.dma_start(out=t[:], in_=d[:])
                krraw = stA.tile([128, S], F16, tag="krraw")

                # --- A1: c_kvT + raw k_rT, one pass over hT s-quarters ---
                for sq in range(4):
                    ps = [psA.tile([128, 512], F32, tag=f"ckv{l}", name=f"ckv{l}")
                          for l in range(L)]
                    kps = psA.tile([128, 512], F32, tag="krp", name="krp")
                    for e in range(E):
                        ht = perm.tile([128, 512], F16, tag="ht", bufs=4)
                        nc.sync.dma_start(
                            out=ht[:],
                            in_=d_hT[e * 128:(e + 1) * 128,
                                     sq * 512:(sq + 1) * 512])
                        for l in range(L):
                            nc.tensor.matmul(
                                ps[l][:], wdkv[:, e, l * 128:(l + 1) * 128],
                                ht[:], start=(e == 0), stop=(e == E - 1))
                        nc.tensor.matmul(kps[:], wkr[:, e, :], ht[:],
                                         start=(e == 0), stop=(e == E - 1))
                    sl = slice(sq * 512, (sq + 1) * 512)
                    for l in range(L):
                        if l % 2 == 0:
                            nc.scalar.activation(
                                out=ckvT[:, l, sl], in_=ps[l][:],
                                func=AF.Identity, bias=bdkv[:, l:l + 1],
                                scale=1.0)
                        else:
                            nc.vector.tensor_scalar_add(
                                ckvT[:, l, sl], ps[l][:], bdkv[:, l:l + 1])
                    nc.vector.tensor_scalar_add(krraw[:, sl], kps[:],
                                                bkr[:])

                # --- A2: rope(k_r): out = raw*cos2 + swap(raw)*sins ---
                krsw = stA.tile([128, S], F16, tag="krsw")
                nc.gpsimd.dma_start(out=krsw[0:64, :], in_=krraw[64:128, :])
                nc.gpsimd.dma_start(out=krsw[64:128, :], in_=krraw[0:64, :])
                nc.vector.tensor_mul(krraw[:], krraw[:], cos2[:])
                nc.vector.tensor_mul(krsw[:], krsw[:], sins[:])
                nc.vector.tensor_add(krro[:], krraw[:], krsw[:])

                # --- A3: c_qT + raw q_rT from resident htq ---
                qps = psA.tile([128, SQ], F32, tag="qrx", name="qrp")
                for e in range(E):
                    nc.tensor.matmul(qps[:], wqr[:, e, :], htq[:, e, :],
                                     start=(e == 0), stop=(e == E - 1))
                qrraw = stA.tile([128, SQ], F16, tag="qrraw")
                nc.scalar.activation(out=qrraw[:], in_=qps[:],
                                     func=AF.Identity, bias=bqr[:], scale=1.0)
                cqT = stA.tile([128, L, SQ], F16)
                for lg in range(2):
                    cps = [psA.tile([128, SQ], F32, tag=f"cq{j}", name=f"cqp{j}")
                           for j in range(2)]
                    for e in range(E):
                        for j in range(2):
                            l = lg * 2 + j
                            nc.tensor.matmul(
                                cps[j][:], wdq[:, e, l * 128:(l + 1) * 128],
                                htq[:, e, :], start=(e == 0), stop=(e == E - 1))
                    for j in range(2):
                        l = lg * 2 + j
                        nc.scalar.activation(out=cqT[:, l, :], in_=cps[j][:],
                                             func=AF.Identity,
                                             bias=bdq[:, l:l + 1], scale=1.0)

                # --- A4: rope(q_r) ---
                qrsw = stA.tile([128, SQ], F16, tag="qrsw")
                nc.gpsimd.dma_start(out=qrsw[0:64, :], in_=qrraw[64:128, :])
                nc.gpsimd.dma_start(out=qrsw[64:128, :], in_=qrraw[0:64, :])
                nc.vector.tensor_mul(qrraw[:], qrraw[:], cos2q[:])
                nc.vector.tensor_mul(qrsw[:], qrsw[:], sinsq[:])
                nc.vector.tensor_add(qrro[:], qrraw[:], qrsw[:])

                # --- A5: q_cT (fp16) = W_UQ^T @ c_qT ---
                for h in range(NH):
                    qp = psA.tile([128, SQ], F32, tag=f"cq{h % 2}", name="qp")
                    for l in range(L):
                        nc.tensor.matmul(qp[:],
                                         wuq[:, l, h * 128:(h + 1) * 128],
                                         cqT[:, l, :], start=(l == 0),
                                         stop=(l == L - 1))
                    nc.scalar.activation(out=qcT[:, h, :], in_=qp[:],
                                         func=AF.Identity,
                                         bias=buq[:, h:h + 1], scale=1.0)

                # --- A6: shared rope scores (scaled, mask+shift folded) ---
                for ks in range(KS):
                    rp = psA.tile([128, SQ], F32, tag=f"cq{ks % 2}", name="rp")
                    nc.tensor.matmul(rp[:], krro[:, ks * 128:(ks + 1) * 128],
                                     qrro[:], start=True, stop=True)
                    nc.scalar.activation(out=ropeS[:, ks, :], in_=rp[:],
                                         func=AF.Identity,
                                         bias=maskb[:, ks:ks + 1], scale=SCALE)

            # =================== Phase B: head groups ===================
            with tc.tile_pool(name="wopool", bufs=1) as woP:
                wotiles = [None] * (2 * E)
                for i in range(0, 2 * E, 2):
                    wt = woP.tile([128, 1024], F16, tag="woA", bufs=2)
                    nh, dh = i // E, i % E
                    nc.sync.dma_start(
                        out=wt[:],
                        in_=d_wo[dh * 128:(dh + 1) * 128,
                                 nh * 1024:(nh + 1) * 1024])
                    wotiles[i] = wt

                with tc.tile_pool(name="stB", bufs=1) as stB, \
                     tc.tile_pool(name="psB", bufs=1, space="PSUM") as psB, \
                     tc.tile_pool(name="drB", bufs=1, space="DRAM") as drB:
                    for g in range(G):
                        if g == G - 1:
                            for i in (1, 3):
                                wt = woP.tile([128, 1024], F16, tag="woB",
                                              bufs=2)
                                nh, dh = i // E, i % E
                                nc.gpsimd.dma_start(
                                    out=wt[:],
                                    in_=d_wo[dh * 128:(dh + 1) * 128,
                                             nh * 1024:(nh + 1) * 1024])
                                wotiles[i] = wt
                        # --- k_cT for 4 heads over full S ---
                        kcT = stB.tile([128, HG, S], F16, tag="kcT")
                        for h4 in range(HG):
                            h = g * HG + h4
                            for sc in range(4):
                                kp = psB.tile([128, 512], F32, tag="work",
                                              bufs=3, name="kp")
                                for l in range(L):
                                    nc.tensor.matmul(
                                        kp[:], wuk[:, l, h * 128:(h + 1) * 128],
                                        ckvT[:, l, sc * 512:(sc + 1) * 512],
                                        start=(l == 0), stop=(l == L - 1))
                                dst = kcT[:, h4, sc * 512:(sc + 1) * 512]
                                if (h4 + sc) % 2 == 0:
                                    nc.scalar.activation(
                                        out=dst, in_=kp[:], func=AF.Identity,
                                        bias=buk[:, h:h + 1], scale=1.0)
                                else:
                                    nc.vector.tensor_scalar_add(
                                        dst, kp[:], buk[:, h:h + 1])
                        # --- v_c for 4 heads over full S (no bias: folded) ---
                        vc = stB.tile([128, KS, HG * 128], F16, tag="vc")
                        for sc2 in range(KS):
                            vp = psB.tile([128, 512], F32, tag="work",
                                          bufs=3, name="vp")
                            for l in range(L):
                                nc.tensor.matmul(
                                    vp[:], ckvT[:, l, sc2 * 128:(sc2 + 1) * 128],
                                    wuv[:, l, g * 512:(g + 1) * 512],
                                    start=(l == 0), stop=(l == L - 1))
                            if sc2 % 2 == 0:
                                nc.scalar.activation(out=vc[:, sc2, :],
                                                     in_=vp[:], func=AF.Copy,
                                                     scale=1.0)
                            else:
                                nc.vector.tensor_copy(vc[:, sc2, :], vp[:])

                        # --- attention for the 4 heads ---
                        op_t = psB.tile([128, SQ], F32, tag="op", bufs=1,
                                        name="opbank")
                        opbank = op_t[0:HG, :]
                        ctxps = []
                        for h4 in range(HG):
                            h = g * HG + h4
                            ctxp = psB.tile([128, SQ], F32, tag="ctx", bufs=4,
                                            name="ctxp")
                            ctxps.append(ctxp)
                            et_aps = [None] * KS
                            spf = None
                            next_sg = [0]
                            yks = [k for k in range(KS) if k not in XKS]
                            ypos = {k: i for i, k in enumerate(yks)}

                            def emit_ctx(ks):
                                nc.tensor.matmul(
                                    ctxp[:],
                                    vc[:, ks, h4 * 128:(h4 + 1) * 128],
                                    et_aps[ks], start=(ks == 0),
                                    stop=(ks == KS - 1))

                            def emit_ready_sums():
                                # softmax sums per 4-chunk subgroup, emitted
                                # once all four exp tiles exist
                                while (next_sg[0] < 4
                                       and et_aps[next_sg[0] * 4 + 3]
                                       is not None):
                                    sg = next_sg[0]
                                    next_sg[0] += 1
                                    k0 = sg * 4
                                    sa = stB.tile([128, SQ], F16, tag="sa",
                                                  bufs=2)
                                    sb = stB.tile([128, SQ], F16, tag="sb",
                                                  bufs=1)
                                    with nc.allow_low_precision(
                                            reason="softmax sums fp16"):
                                        nc.gpsimd.tensor_add(
                                            sa[:], et_aps[k0], et_aps[k0 + 1])
                                        nc.gpsimd.tensor_add(
                                            sb[:], et_aps[k0 + 2],
                                            et_aps[k0 + 3])
                                    nc.tensor.matmul(
                                        opbank, sel4[:, h4, :], sa[:],
                                        start=(h4 == 0 and sg == 0),
                                        stop=False)
                                    nc.tensor.matmul(
                                        opbank, sel4[:, h4, :], sb[:],
                                        start=False,
                                        stop=(h4 == HG - 1 and sg == 3))

                            for ks in range(KS):
                                sp = psB.tile([128, SQ], F32, tag="work",
                                              bufs=3, name="sp")
                                nc.tensor.matmul(
                                    sp[:], kcT[:, h4, ks * 128:(ks + 1) * 128],
                                    qcT[:, h, :], start=True,
                                    stop=(ks not in XKS))
                                if ks in XKS:
                                    nc.tensor.matmul(
                                        sp[:], krro[:, ks * 128:(ks + 1) * 128],
                                        qrro[:], start=False, stop=True)
                                    et1 = stB.tile([128, SQ], F16, tag="et1",
                                                   bufs=2)
                                    nc.scalar.activation(
                                        out=et1[:], in_=sp[:], func=AF.Exp,
                                        bias=maskb[:, ks:ks + 1], scale=SCALE)
                                    et_aps[ks] = et1[:]
                                    emit_ctx(ks)
                                    emit_ready_sums()
                                else:
                                    p = ypos[ks] % 2
                                    if p == 0:
                                        spf = stB.tile([128, 2, SQ], F16,
                                                       tag="spf", bufs=2)
                                    nc.vector.scalar_tensor_tensor(
                                        out=spf[:, p, :], in0=sp[:],
                                        scalar=SCALE, in1=ropeS[:, ks, :],
                                        op0=ALU.mult, op1=ALU.add)
                                    if p == 1:
                                        kprev = yks[ypos[ks] - 1]
                                        et2 = stB.tile([128, 2, SQ], F16,
                                                       tag="et2", bufs=3)
                                        nc.scalar.activation(
                                            out=et2[:], in_=spf[:],
                                            func=AF.Exp, bias=0.0, scale=1.0)
                                        et_aps[kprev] = et2[:, 0, :]
                                        et_aps[ks] = et2[:, 1, :]
                                        emit_ctx(kprev)
                                        emit_ctx(ks)
                                        emit_ready_sums()

                        # --- group normalize: ctxf = ctxp / sums ---
                        sums4 = stB.tile([HG, SQ], F32, tag="sums4", bufs=1)
                        nc.vector.tensor_copy(sums4[:], opbank)
                        rsum4 = stB.tile([HG, SQ], F16, tag="rsum4", bufs=1)
                        with nc.allow_low_precision(reason="softmax recip f16"):
                            nc.vector.reciprocal(rsum4[:], sums4[:])
                        for h4 in range(HG):
                            h = g * HG + h4
                            rp = psB.tile([128, SQ], F32, tag="work", bufs=3,
                                          name="rbp")
                            nc.tensor.matmul(rp[:], selb[0:4, h4, :],
                                             rsum4[:], start=True, stop=True)
                            rb = stB.tile([128, SQ], F16, tag="rb", bufs=2)
                            nc.scalar.activation(out=rb[:], in_=rp[:],
                                                 func=AF.Copy, scale=1.0)
                            nc.vector.tensor_mul(ctxf[:, h, :], ctxps[h4][:],
                                                 rb[:])

                # =================== Phase C: W_O ===================
                with tc.tile_pool(name="stC", bufs=1) as stC, \
                     tc.tile_pool(name="psC", bufs=1, space="PSUM") as psC:
                    for i in range(5, 2 * E, 2):
                        wt = woP.tile([128, 1024], F16, tag="woB", bufs=2)
                        nh, dh = i // E, i % E
                        nc.gpsimd.dma_start(
                            out=wt[:],
                            in_=d_wo[dh * 128:(dh + 1) * 128,
                                     nh * 1024:(nh + 1) * 1024])
                        wotiles[i] = wt
                    for nh in range(2):
                        ops = [[psC.tile([128, 512], F32, tag=f"o{q4}{n2}",
                                         name=f"o{q4}{n2}")
                                for n2 in range(2)] for q4 in range(4)]
                        for dh in range(E):
                            wo = wotiles[nh * E + dh]
                            for q4 in range(4):
                                for n2 in range(2):
                                    nc.tensor.matmul(
                                        ops[q4][n2][:],
                                        ctxf[:, dh, q4 * 128:(q4 + 1) * 128],
                                        wo[:, n2 * 512:(n2 + 1) * 512],
                                        start=(dh == 0), stop=(dh == E - 1))
                        for q4 in range(4):
                            ot = stC.tile([128, 1024], F32, tag="ot", bufs=4)
                            for n2 in range(2):
                                sl = slice(nh * 1024 + n2 * 512,
                                           nh * 1024 + (n2 + 1) * 512)
                                nc.vector.tensor_add(
                                    ot[:, n2 * 512:(n2 + 1) * 512],
                                    ops[q4][n2][:], bo_b[:, sl])
                            nc.scalar.dma_start(
                                out=d_out[q4 * 128:(q4 + 1) * 128,
                                          nh * 1024:(nh + 1) * 1024],
                                in_=ot[:])
    _split_multi_waits(nc)
    return nc


_cache = {}


def _get_nc():
    if "nc" not in _cache:
        _cache["nc"] = build()
    return _cache["nc"]


def _host_prep(hidden_states, attention_mask, W_DKV, b_DKV, W_DQ, b_DQ,
               W_UK, b_UK, W_UV, b_UV, W_UQ, b_UQ,
               W_KR, b_KR, W_QR, b_QR, W_O, b_O):
    f32 = np.float32
    f16 = np.float16
    hidden = np.asarray(hidden_states, f32)
    mask = np.asarray(attention_mask)

    inv = 1.0 / (10000.0 ** (np.arange(0, D, 2, dtype=np.float64) / D))
    ang = inv[:, None] * np.arange(S, dtype=np.float64)[None, :]   # [64, S]
    cos = np.cos(ang)
    sin = np.sin(ang)
    cos2 = np.concatenate([cos, cos], 0).astype(f16)      # [128, S]
    sins = np.concatenate([-sin, sin], 0).astype(f16)

    wo_f32 = np.asarray(W_O, f32)
    bo_eff = (np.asarray(b_O, f32)
              + np.asarray(b_UV, f32) @ wo_f32).astype(f32)

    shared = {
        "wdkv": np.ascontiguousarray(np.asarray(W_DKV, f32).astype(f16)),
        "wdq": np.ascontiguousarray(np.asarray(W_DQ, f32).astype(f16)),
        "wkr": np.ascontiguousarray(np.asarray(W_KR, f32).astype(f16)),
        "wqr": np.ascontiguousarray(np.asarray(W_QR, f32).astype(f16)),
        "wuk": np.ascontiguousarray(np.asarray(W_UK, f32).astype(f16)),
        "wuv": np.ascontiguousarray(np.asarray(W_UV, f32).astype(f16)),
        "wuq": np.ascontiguousarray(np.asarray(W_UQ, f32).astype(f16)),
        "wo": np.ascontiguousarray(wo_f32.astype(f16)),
        "cos2": np.ascontiguousarray(cos2),
        "sins": np.ascontiguousarray(sins),
        "bdkv": np.ascontiguousarray(np.asarray(b_DKV, f32).reshape(L, 128).T),
        "bdq": np.ascontiguousarray(np.asarray(b_DQ, f32).reshape(L, 128).T),
        "buk": np.ascontiguousarray(np.asarray(b_UK, f32).reshape(NH, 128).T),
        "buq": np.ascontiguousarray(np.asarray(b_UQ, f32).reshape(NH, 128).T),
        "bkr": np.asarray(b_KR, f32).reshape(128, 1),
        "bqr": np.asarray(b_QR, f32).reshape(128, 1),
        "bo": np.ascontiguousarray(bo_eff.reshape(1, H).astype(f16)),
        "sel4": np.ascontiguousarray(
            np.tile(np.eye(4, dtype=f16), (128, 1)).reshape(128, 16)),
        "selb": np.ascontiguousarray(
            np.kron(np.eye(4, dtype=f16), np.ones((1, 128), f16))),
    }
    per_batch = {}
    for b in range(B):
        hT = np.ascontiguousarray(hidden[b].T.astype(f16))
        mb = np.where(np.asarray(mask[b]) == 0, -1e30, 0.0).astype(f32) - SHIFT
        per_batch[b] = {
            "hT": hT,
            "maskb": np.ascontiguousarray(mb.reshape(E, 128).T),
        }
    in_maps = []
    for core in range(8):
        b, qc = core // 4, core % 4
        qsl = slice(qc * SQ, (qc + 1) * SQ)
        m = dict(shared)
        m.update(per_batch[b])
        m["hTq"] = np.ascontiguousarray(per_batch[b]["hT"][:, qsl])
        m["cos2q"] = np.ascontiguousarray(cos2[:, qsl])
        m["sinsq"] = np.ascontiguousarray(sins[:, qsl])
        in_maps.append(m)
    return in_maps


def kernel(**inputs):
    nc = _get_nc()
    in_maps = _host_prep(**inputs)
    res = run_bass_kernel_spmd(nc, in_maps, list(range(8)))
    out = np.empty((B, S, H), np.float32)
    for core in range(8):
        b, qc = core // 4, core % 4
        out[b, qc * SQ:(qc + 1) * SQ, :] = res.results[core]["out"]
    return out
